# revision 1
# baseline (speedup 1.0000x reference)
"""Causal self-attention on 8 TRN2 NeuronCores.

Problem: B=4, T=2048, C=1024, NH=16, HD=64.
  qkv = x @ w_qkv ; per-head causal softmax attention ; y @ w_proj

Sharding (data+tensor parallel, host gather instead of a final all-reduce):
  - heads split 2-per-core for qkv + attention (each core gets its 128
    columns of w_q/w_k/w_v),
  - x is PE-transposed on the fly inside the qkv loop (feature-major
    activations throughout),
  - attention runs fully feature-major (q^T/k^T, v-with-ones trick so the
    softmax denominator falls out of the same matmul),
  - y^T shards are exchanged with an AllToAll so each core projects its
    own 1/8 of the rows through the full w_proj,
  - host reassembles the 8 [1024 oc, 1024 rows] output slices.

All matmuls run in float32r (TF32-like, full PE rate at free-dim>=256),
~3e-4 relative error vs fp32.
"""

import numpy as np

import concourse.bass as bass
import concourse.mybir as mybir
import concourse.tile as tile
from concourse import bacc
from concourse.bass_utils import run_bass_kernel_spmd
from concourse.masks import make_identity

B, T, C = 4, 2048, 1024
NH, HD = 16, 64
NCORES = 8
HPC = NH // NCORES          # heads per core = 2
D2 = HPC * HD               # 128 head-dims per core
ROWS = B * T                # 8192 flattened rows
RSL = ROWS // NCORES        # 1024 rows per core slice
P = 128
QTL = 512                   # q tile (free dim of score matmuls)
NJ = T // QTL               # 4 q-tiles per batch
KVC = T // P                # 16 kv chunks of 128 per batch
NCH = ROWS // QTL           # 16 row-chunks for qkv
CK = C // P                 # 8 contraction chunks
SCALE = 1.0 / np.sqrt(HD)

F32 = mybir.dt.float32
F32R = mybir.dt.float32r
AF = mybir.ActivationFunctionType
ALU = mybir.AluOpType

_CACHED_NC = None
LAST_RESULTS = None  # BassKernelResults of the most recent launch (for profiling)

try:  # reuse compiled executables across calls/processes when supported
    import jax

    jax.config.update("jax_compilation_cache_dir", "/tmp/jax_cache")
    jax.config.update("jax_persistent_cache_min_compile_time_secs", 1.0)
except Exception:
    pass


def _build_nc():
    nc = bacc.Bacc(None, target_bir_lowering=False, num_devices=NCORES)

    x_in = nc.dram_tensor("x_full", [ROWS, C], F32, kind="ExternalInput")
    wq = nc.dram_tensor("wq", [C, D2], F32, kind="ExternalInput")
    wk = nc.dram_tensor("wk", [C, D2], F32, kind="ExternalInput")
    wv = nc.dram_tensor("wv", [C, D2], F32, kind="ExternalInput")
    wp = nc.dram_tensor("wp", [C, C], F32, kind="ExternalInput")
    mkp = nc.dram_tensor("maskp", [4, P, QTL], F32, kind="ExternalInput")
    outT = nc.dram_tensor("outT", [C, RSL], F32, kind="ExternalOutput")

    rg = [list(range(NCORES))]

    with tile.TileContext(nc) as tc:
        with (
            tc.tile_pool(name="persist", bufs=1) as pp,
            tc.tile_pool(name="dram", bufs=1, space="DRAM") as dram,
        ):
            # ---- all DRAM collective buffers up-front (no slot reuse) ----
            # two half-buffers (even/odd q-tiles) so the first AllToAll can
            # overlap the attention tail
            a2a_in0 = dram.tile([NCORES, D2, RSL // 2], F32R)
            a2a_in1 = dram.tile([NCORES, D2, RSL // 2], F32R)
            a2a_out0 = dram.tile([NCORES, D2, RSL // 2], F32R)
            a2a_out1 = dram.tile([NCORES, D2, RSL // 2], F32R)

            ident = pp.tile([P, P], F32)
            make_identity(nc, ident[:])

            # mask patterns (multiplicative, diagonal blocks), fp32
            mask_sb = pp.tile([P, 4, QTL], F32)
            for m in range(4):
                nc.sync.dma_start(mask_sb[:, m, :], mkp[m])

            # persistent SBUF activations
            qT = pp.tile([P, ROWS], F32R)    # 2 heads x 64 dims on partitions
            kT = pp.tile([P, ROWS], F32R)
            vaug = pp.tile([P, KVC * B, 2 * HD + 2], F32R)  # [kv, chunk, v|1|v|1]

            # ones columns of vaug
            ones_f = pp.tile([P, KVC * B], F32)
            nc.vector.memset(ones_f[:], 1.0)
            nc.vector.tensor_copy(vaug[:, :, HD], ones_f[:])
            nc.vector.tensor_copy(vaug[:, :, 2 * HD + 1], ones_f[:])

            # ones row (partition HD) for the K=1 denominator broadcast matmul
            ones_r = pp.tile([P, HD], F32R)
            nc.vector.tensor_copy(ones_r[:], ones_f[:, 0:HD])

            # qkv weights -> f32r [P, CK, D2]
            w_r = {}
            with tc.tile_pool(name="wload", bufs=2) as pW:
                for nm, wt in (("q", wq), ("k", wk), ("v", wv)):
                    wr = pp.tile([P, CK, D2], F32R, name=f"w_{nm}")
                    for ko in range(CK):
                        wf = pW.tile([P, D2], F32, tag="wf", bufs=3)
                        nc.sync.dma_start(wf[:], wt[ko * P : (ko + 1) * P, :])
                        nc.vector.tensor_copy(wr[:, ko, :], wf[:])
                    w_r[nm] = wr

            # ---------------- phase B: x-transpose + qkv + v transpose ----------------
            with (
                tc.tile_pool(name="phB", bufs=2) as pB,
                tc.tile_pool(name="psB", bufs=2, space="PSUM") as psB,
            ):
                for n in range(NCH):
                    # load 512 rows of x, transpose to feature-major tiles
                    xt_tiles = []
                    for co in range(CK):
                        xt = pB.tile(
                            [P, QTL], F32R, tag=f"xt{co}", bufs=2, name=f"xt{co}"
                        )
                        xt_tiles.append(xt)
                    for ro in range(QTL // P):
                        x_t = pB.tile([P, C], F32, tag="x_t", bufs=4)
                        nc.sync.dma_start(
                            x_t[:], x_in[n * QTL + ro * P : n * QTL + (ro + 1) * P, :]
                        )
                        for co in range(CK):
                            ps_x = psB.tile([P, P], F32, tag="ps_x", bufs=2)
                            nc.tensor.transpose(
                                ps_x[:], x_t[:, co * P : (co + 1) * P], ident[:]
                            )
                            dst = xt_tiles[co][:, ro * P : (ro + 1) * P]
                            if co % 2 == 0:
                                nc.vector.tensor_copy(dst, ps_x[:])
                            else:
                                nc.scalar.copy(dst, ps_x[:])
                    ps_q = psB.tile([P, QTL], F32, tag="ps_q", bufs=2)
                    ps_k = psB.tile([P, QTL], F32, tag="ps_k", bufs=2)
                    ps_v = psB.tile([P, QTL], F32, tag="ps_v", bufs=2)
                    for ko in range(CK):
                        st = ko == 0
                        sp = ko == CK - 1
                        nc.tensor.matmul(ps_q[:], w_r["q"][:, ko, :], xt_tiles[ko][:], start=st, stop=sp)
                        nc.tensor.matmul(ps_k[:], w_r["k"][:, ko, :], xt_tiles[ko][:], start=st, stop=sp)
                        nc.tensor.matmul(ps_v[:], w_r["v"][:, ko, :], xt_tiles[ko][:], start=st, stop=sp)
                    nsl = slice(n * QTL, (n + 1) * QTL)
                    nc.scalar.activation(qT[:, nsl], ps_q[:], AF.Copy, scale=float(SCALE))
                    nc.scalar.copy(kT[:, nsl], ps_k[:])
                    vt_tmp = pB.tile([P, QTL], F32, tag="vt_tmp", bufs=2)
                    nc.scalar.copy(vt_tmp[:], ps_v[:])
                    for sub in range(QTL // P):
                        ch = 4 * n + sub
                        ps_tv = psB.tile([P, P], F32, tag="ps_x", bufs=2)
                        nc.tensor.transpose(
                            ps_tv[:], vt_tmp[:, sub * P : (sub + 1) * P], ident[:]
                        )
                        nc.vector.tensor_copy(vaug[:, ch, 0:HD], ps_tv[:, 0:HD])
                        nc.vector.tensor_copy(
                            vaug[:, ch, HD + 1 : 2 * HD + 1], ps_tv[:, HD : 2 * HD]
                        )

            # ---------------- phase C: attention ----------------
            with (
                tc.tile_pool(name="phC", bufs=2) as pC,
                tc.tile_pool(name="psC", bufs=2, space="PSUM") as psC,
            ):
                for b in range(B):
                    for j in range(NJ):
                        qsl = slice((b * NJ + j) * QTL, (b * NJ + j + 1) * QTL)
                        nkv = 4 * j + 4
                        ps_ys = [
                            psC.tile(
                                [HD + 1, QTL], F32, tag=f"ps_y{h}", bufs=1,
                                name=f"ps_y{h}",
                            )
                            for h in range(HPC)
                        ]
                        for i in range(nkv):
                            ch = b * KVC + i
                            m = i - 4 * j
                            atts = []
                            ps_ss = []
                            for h in range(HPC):
                                hsl = slice(h * HD, (h + 1) * HD)
                                ps_s = psC.tile([P, QTL], F32, tag=f"ps_s{h}", bufs=2)
                                # both heads issue back-to-back: different PE
                                # row groups run concurrently
                                nc.tensor.matmul(
                                    ps_s[:],
                                    kT[hsl, ch * P : (ch + 1) * P],
                                    qT[hsl, qsl],
                                    start=True, stop=True,
                                )
                                ps_ss.append(ps_s)
                            for h in range(HPC):
                                att = pC.tile([P, QTL], F32R, tag=f"att{h}", bufs=3)
                                nc.scalar.activation(att[:], ps_ss[h][:], AF.Exp)
                                if m >= 0:
                                    nc.vector.tensor_tensor(
                                        att[:], att[:], mask_sb[:, m, :], ALU.mult
                                    )
                                atts.append(att)
                            for h in range(HPC):
                                vsl = slice(h * (HD + 1), (h + 1) * (HD + 1))
                                nc.tensor.matmul(
                                    ps_ys[h][:],
                                    vaug[:, ch, vsl],
                                    atts[h][:],
                                    start=(i == 0), stop=(i == nkv - 1),
                                )
                        for h in range(HPC):
                            ps_y = ps_ys[h]
                            # normalize rows 0:HD by row HD (all on ACT + PE):
                            # 1/s computed as exp(-ln s) (ACT Reciprocal is banned)
                            ln_t = pC.tile([HD + 1, QTL], F32, tag=f"ln{h}", bufs=2)
                            nc.scalar.activation(
                                ln_t[HD : HD + 1, :], ps_y[HD : HD + 1, :], AF.Ln
                            )
                            rec = pC.tile([HD + 1, QTL], F32R, tag=f"rec{h}", bufs=2)
                            nc.scalar.activation(
                                rec[HD : HD + 1, :], ln_t[HD : HD + 1, :],
                                AF.Exp, scale=-1.0,
                            )
                            ps_bc = psC.tile([HD, QTL], F32, tag="ps_bc", bufs=2)
                            nc.tensor.matmul(
                                ps_bc[:],
                                ones_r[HD : HD + 1, :],
                                rec[HD : HD + 1, :],
                                start=True, stop=True,
                            )
                            yraw = pC.tile([HD, QTL], F32, tag=f"yraw{h}", bufs=2)
                            nc.scalar.copy(yraw[:], ps_y[0:HD, :])
                            ynrm = pC.tile([HD, QTL], F32R, tag=f"ynrm{h}", bufs=2)
                            nc.vector.tensor_tensor(ynrm[:], yraw[:], ps_bc[:], ALU.mult)
                            row0 = b * T + j * QTL
                            s = row0 // RSL
                            buf = a2a_in0 if (row0 % RSL) == 0 else a2a_in1
                            nc.sync.dma_start(
                                buf[s, h * HD : (h + 1) * HD, :], ynrm[:]
                            )

            nc.gpsimd.collective_compute(
                "AllToAll", ALU.bypass, replica_groups=rg,
                ins=[a2a_in0[:].opt()], outs=[a2a_out0[:].opt()],
            )
            nc.gpsimd.collective_compute(
                "AllToAll", ALU.bypass, replica_groups=rg,
                ins=[a2a_in1[:].opt()], outs=[a2a_out1[:].opt()],
            )

            # ---------------- phase D: projection for own row slice ----------------
            with (
                tc.tile_pool(name="phD", bufs=2) as pD,
                tc.tile_pool(name="psD", bufs=2, space="PSUM") as psD,
            ):
                wpr = pp.tile([P, CK, C], F32R)
                for ko in range(CK):
                    wpf = pD.tile([P, C], F32, tag="wpf", bufs=2)
                    nc.sync.dma_start(wpf[:], wp[ko * P : (ko + 1) * P, :])
                    nc.vector.tensor_copy(wpr[:, ko, :], wpf[:])
                for s2 in range(RSL // QTL):
                    a2a_out = a2a_out0 if s2 == 0 else a2a_out1
                    yr_tiles = []
                    for kk in range(NCORES):
                        yr = pD.tile([P, QTL], F32R, tag="yr", bufs=2 * NCORES + 2)
                        nc.sync.dma_start(yr[:], a2a_out[kk, :, :])
                        yr_tiles.append(yr)
                    for oc in range(CK):
                        ps_o = psD.tile([P, QTL], F32, tag="ps_o", bufs=2)
                        for kk in range(NCORES):
                            nc.tensor.matmul(
                                ps_o[:],
                                wpr[:, kk, oc * P : (oc + 1) * P],
                                yr_tiles[kk][:],
                                start=(kk == 0), stop=(kk == NCORES - 1),
                            )
                        osb = pD.tile([P, QTL], F32, tag="osb", bufs=3)
                        nc.scalar.copy(osb[:], ps_o[:])
                        nc.sync.dma_start(
                            outT[oc * P : (oc + 1) * P, s2 * QTL : (s2 + 1) * QTL],
                            osb[:],
                        )

    nc.finalize()
    return nc


def _get_nc():
    global _CACHED_NC
    if _CACHED_NC is None:
        _CACHED_NC = _build_nc()
    return _CACHED_NC


def kernel(x, mask, w_qkv, w_proj):
    x = np.ascontiguousarray(np.asarray(x, dtype=np.float32))
    mask = np.asarray(mask)
    w_qkv = np.ascontiguousarray(np.asarray(w_qkv, dtype=np.float32))
    w_proj = np.ascontiguousarray(np.asarray(w_proj, dtype=np.float32))

    xf = np.ascontiguousarray(x.reshape(ROWS, C))
    # transposed diagonal-block mask patterns, multiplicative
    mt = mask.reshape(T, T).T.astype(np.float32)
    maskp = np.stack([mt[m * P : m * P + P, 0:QTL] for m in range(4)])
    maskp = np.ascontiguousarray(maskp)

    in_maps = []
    for r in range(NCORES):
        in_maps.append(
            {
                "x_full": xf,
                "wq": np.ascontiguousarray(w_qkv[:, r * D2 : (r + 1) * D2]),
                "wk": np.ascontiguousarray(w_qkv[:, C + r * D2 : C + (r + 1) * D2]),
                "wv": np.ascontiguousarray(
                    w_qkv[:, 2 * C + r * D2 : 2 * C + (r + 1) * D2]
                ),
                "wp": w_proj,
                "maskp": maskp,
            }
        )

    nc = _get_nc()
    res = run_bass_kernel_spmd(nc, in_maps, core_ids=list(range(NCORES)))
    global LAST_RESULTS
    LAST_RESULTS = res

    out = np.empty((ROWS, C), dtype=np.float32)
    for r in range(NCORES):
        out[r * RSL : (r + 1) * RSL, :] = res.results[r]["outT"].T
    return out.reshape(B, T, C)



# revision 14
# speedup vs baseline: 1.4317x; 1.4317x over previous
"""Causal self-attention on 8 TRN2 NeuronCores.

Problem: B=4, T=2048, C=1024, NH=16, HD=64.
  qkv = x @ w_qkv ; per-head causal softmax attention ; y @ w_proj

Sharding: heads 2-per-core for qkv+attention; AllToAll of y^T shards so
each core projects 2 of the 16 512-row tiles through the full w_proj;
host reassembles.

Engine plan (vs. the fp32r baseline):
  - bf16 datapath for x/w/q/k/v/att (host casts; scale 1/8 folded into wq),
    f32r for the y/proj path (no on-chip dtype conversions there).
  - Act engine does exp ONLY (plus nothing else): softmax denominators are
    reciprocal'd on DVE (reciprocal_approx_fast), killing the Ln/Exp trick
    and its ACT_TABLE_LOAD storms.  Both heads' scores land in one 2-bank
    PSUM tile so one exp covers [128, 1024].
  - qkv chunk n is emitted fine-grained-interleaved with attention tile
    n-1, so the PE always has independent work while Act chews exps, and
    stays in its 2.4 GHz p-state.
  - AllToAll split in two (tiles 0-7 / 8-15); the first fires at 50% of
    attention and its projection interleaves with the last attention tile.
"""

import numpy as np
import ml_dtypes

import concourse.bass as bass
import concourse.mybir as mybir
import concourse.tile as tile
from concourse import bacc
from concourse.bass_utils import run_bass_kernel_spmd
from concourse.masks import make_identity

B, T, C = 4, 2048, 1024
NH, HD = 16, 64
NCORES = 8
HPC = NH // NCORES          # heads per core = 2
D2 = HPC * HD               # 128 head-dims per core
ROWS = B * T                # 8192 flattened rows
P = 128
QTL = 512                   # q tile / row-chunk size
NT = ROWS // QTL            # 16 tiles; tile t = rows t*512..(t+1)*512
CK = C // P                 # 8 contraction chunks
KVC = T // P                # 16 kv chunks per batch
SCALE = 1.0 / np.sqrt(HD)

F32 = mybir.dt.float32
F32R = mybir.dt.float32r
BF16 = mybir.dt.bfloat16
AF = mybir.ActivationFunctionType
ALU = mybir.AluOpType

_CACHED_NC = None
LAST_RESULTS = None

try:
    import jax

    jax.config.update("jax_compilation_cache_dir", "/tmp/jax_cache")
    jax.config.update("jax_persistent_cache_min_compile_time_secs", 1.0)
except Exception:
    pass


def _interleave(a, b):
    """Merge two unit lists proportionally (a paced across b)."""
    if not a:
        return list(b)
    if not b:
        return list(a)
    out = []
    na, nb = len(a), len(b)
    ia = ib = 0
    while ia < na or ib < nb:
        # emit whichever list is behind proportionally
        if ib >= nb or (ia < na and ia * nb <= ib * na):
            out.append(a[ia]); ia += 1
        else:
            out.append(b[ib]); ib += 1
    return out


def _build_nc():
    nc = bacc.Bacc(None, target_bir_lowering=False, num_devices=NCORES)

    x_in = nc.dram_tensor("x_full", [ROWS, C], BF16, kind="ExternalInput")
    wq = nc.dram_tensor("wq", [C, D2], BF16, kind="ExternalInput")
    wk = nc.dram_tensor("wk", [C, D2], BF16, kind="ExternalInput")
    wv = nc.dram_tensor("wv", [C, D2], BF16, kind="ExternalInput")
    wp = nc.dram_tensor("wp", [C, C], F32R, kind="ExternalInput")
    mk = nc.dram_tensor("mask2", [4, P, 2 * QTL], BF16, kind="ExternalInput")
    outT = nc.dram_tensor("outT", [C, 2 * QTL], F32, kind="ExternalOutput")

    rg = [list(range(NCORES))]

    with tile.TileContext(nc) as tc:
        with (
            tc.tile_pool(name="persist", bufs=1) as pp,
            tc.tile_pool(name="dram", bufs=1, space="DRAM") as dram,
            tc.tile_pool(name="phB", bufs=2) as pB,
            tc.tile_pool(name="phC", bufs=2) as pC,
            tc.tile_pool(name="phD", bufs=2) as pD,
            tc.tile_pool(name="psum", bufs=1, space="PSUM") as psp,
        ):
            # ---- DRAM collective buffers ----
            a2a_in = [
                dram.tile([NCORES, D2, QTL], F32R, name=f"a2a_in{i}")
                for i in range(2)
            ]
            a2a_out = [
                dram.tile([NCORES, D2, QTL], F32R, name=f"a2a_out{i}")
                for i in range(2)
            ]

            # ---- persistent SBUF ----
            ident = pp.tile([P, P], BF16)
            make_identity(nc, ident[:])

            mask_sb = pp.tile([P, 4, 2 * QTL], BF16)
            nc.sync.dma_start(
                mask_sb[:], mk[:].rearrange("m p q -> p m q")
            )

            qT = pp.tile([P, ROWS], BF16)    # 2 heads x 64 dims on partitions
            kT = pp.tile([P, ROWS], BF16)
            vaug = pp.tile([P, B * KVC, 2 * HD + 2], BF16)  # [kv, ch, v0|1|v1|1]

            nc.vector.memset(vaug[:, :, HD], 1.0)
            nc.vector.memset(vaug[:, :, 2 * HD + 1], 1.0)

            ones_f = pp.tile([P, HD], F32)
            nc.vector.memset(ones_f[:], 1.0)
            ones_hd = pp.tile([P, HD], F32R)
            nc.vector.tensor_copy(ones_hd[:], ones_f[:])

            w_r = {}
            for nm, wt in (("q", wq), ("k", wk), ("v", wv)):
                wr = pp.tile([P, CK, D2], BF16, name=f"w_{nm}")
                nc.sync.dma_start(
                    wr[:], wt[:].rearrange("(ko p) d -> p ko d", p=P)
                )
                w_r[nm] = wr
            wpr = pp.tile([P, CK, C], F32R)

            # =====================================================
            # unit builders
            # =====================================================
            def b_units(n):
                """qkv for row chunk n (512 rows)."""
                units = []
                x_t = pB.tile([P, 4, C], BF16, tag="x_t", bufs=2)
                xts = pB.tile([P, CK, QTL], BF16, tag="xts", bufs=2)

                def u_dma():
                    nc.sync.dma_start(
                        x_t[:],
                        x_in[n * QTL : (n + 1) * QTL, :].rearrange(
                            "(ro p) c -> p ro c", p=P
                        ),
                    )
                units.append(u_dma)

                for ro in range(4):
                    def u_tr(ro=ro):
                        ps_xt = psp.tile([P, CK * P], BF16, tag="xt", bufs=2)
                        for co in range(CK):
                            nc.tensor.transpose(
                                ps_xt[:, co * P : (co + 1) * P],
                                x_t[:, ro, co * P : (co + 1) * P],
                                ident[:],
                            )
                        # strided scatter into [c-part, ko, rows]
                        nc.vector.tensor_copy(
                            xts[:, :, ro * P : (ro + 1) * P],
                            ps_xt[:].rearrange("p (co r) -> p co r", co=CK),
                        )
                    units.append(u_tr)

                ps_qk = {}

                def u_qk0():
                    ps_qk["q"] = psp.tile([P, QTL], F32, tag="qkv", bufs=2, name="ps_q")
                    ps_qk["k"] = psp.tile([P, QTL], F32, tag="qkv", bufs=2, name="ps_k")
                    for ko in range(2):
                        for nm in ("q", "k"):
                            nc.tensor.matmul(
                                ps_qk[nm][:], w_r[nm][:, ko, :], xts[:, ko, :],
                                start=(ko == 0), stop=False,
                            )
                units.append(u_qk0)
                for kg in range(1, 4):
                    def u_qk(kg=kg):
                        for ko in range(2 * kg, 2 * kg + 2):
                            for nm in ("q", "k"):
                                nc.tensor.matmul(
                                    ps_qk[nm][:], w_r[nm][:, ko, :], xts[:, ko, :],
                                    start=False, stop=(ko == CK - 1),
                                )
                    units.append(u_qk)

                nsl = slice(n * QTL, (n + 1) * QTL)

                def u_qkcopy():
                    nc.vector.tensor_copy(qT[:, nsl], ps_qk["q"][:])
                    nc.vector.tensor_copy(kT[:, nsl], ps_qk["k"][:])
                units.append(u_qkcopy)

                vt_tmp = pB.tile([P, QTL], BF16, tag="vt", bufs=2)

                def u_v1():
                    ps_v = psp.tile([P, QTL], F32, tag="qkv", bufs=2)
                    for ko in range(CK):
                        nc.tensor.matmul(
                            ps_v[:], w_r["v"][:, ko, :], xts[:, ko, :],
                            start=(ko == 0), stop=(ko == CK - 1),
                        )
                    nc.vector.tensor_copy(vt_tmp[:], ps_v[:])
                units.append(u_v1)

                def u_v2():
                    ps_vt = psp.tile([P, CK * P], BF16, tag="xt", bufs=2)
                    for sub in range(4):
                        nc.tensor.transpose(
                            ps_vt[:, sub * P : (sub + 1) * P],
                            vt_tmp[:, sub * P : (sub + 1) * P],
                            ident[:],
                        )
                    ch0 = 4 * n
                    pv = ps_vt[:, 0 : 4 * P].rearrange("p (s d) -> p s d", s=4)
                    nc.vector.tensor_copy(vaug[:, ch0 : ch0 + 4, 0:HD], pv[:, :, 0:HD])
                    nc.vector.tensor_copy(
                        vaug[:, ch0 : ch0 + 4, HD + 1 : 2 * HD + 1],
                        pv[:, :, HD : 2 * HD],
                    )
                units.append(u_v2)
                return units

            def c_units(t):
                """attention for tile t (b=t//4, j=t%4)."""
                b, j = t // 4, t % 4
                nkv = 4 * j + 4
                qsl = slice(t * QTL, (t + 1) * QTL)
                units = []
                atts = [None] * nkv
                ps_ys = {}

                def u_y0():
                    ps_ys[0] = psp.tile([HD + 1, QTL], F32, tag="y0", bufs=1, name="ps_y0")
                    ps_ys[1] = psp.tile([HD + 1, QTL], F32, tag="y1", bufs=1, name="ps_y1")
                units.append(u_y0)

                for i in range(nkv):
                    def u_s(i=i):
                        ch = b * KVC + i
                        m = i - 4 * j
                        ps_s = psp.tile([P, 2 * QTL], F32, tag="s", bufs=1)
                        for h in range(HPC):
                            hsl = slice(h * HD, (h + 1) * HD)
                            nc.tensor.matmul(
                                ps_s[:, h * QTL : (h + 1) * QTL],
                                kT[hsl, ch * P : (ch + 1) * P],
                                qT[hsl, qsl],
                                start=True, stop=True,
                            )
                        att = pC.tile([P, 2 * QTL], BF16, tag="att", bufs=18)
                        for hh in range(HPC):
                            hq = slice(hh * QTL, (hh + 1) * QTL)
                            nc.scalar.activation(att[:, hq], ps_s[:, hq], AF.Exp)
                        if m >= 0:
                            nc.vector.tensor_tensor(
                                att[:], att[:], mask_sb[:, m, :], ALU.mult
                            )
                        atts[i] = att
                    units.append(u_s)

                for i in range(nkv):
                    def u_av(i=i):
                        ch = b * KVC + i
                        for h in range(HPC):
                            vsl = slice(h * (HD + 1), (h + 1) * (HD + 1))
                            nc.tensor.matmul(
                                ps_ys[h][:],
                                vaug[:, ch, vsl],
                                atts[i][:, h * QTL : (h + 1) * QTL],
                                start=(i == 0), stop=(i == nkv - 1),
                            )
                    units.append(u_av)

                for h in range(HPC):
                    def u_tail(h=h):
                        ps_y = ps_ys[h]
                        rec = pC.tile([HD + 1, QTL], F32, tag="rec", bufs=2)
                        nc.vector.reciprocal(
                            rec[HD : HD + 1, :], ps_y[HD : HD + 1, :]
                        )
                        recr = pC.tile([HD + 1, QTL], F32R, tag="recr", bufs=2)
                        nc.vector.tensor_copy(
                            recr[HD : HD + 1, :], rec[HD : HD + 1, :]
                        )
                        ps_bc = psp.tile([P, 2 * QTL], F32, tag="s", bufs=1)
                        nc.tensor.matmul(
                            ps_bc[0:HD, h * QTL : (h + 1) * QTL],
                            ones_hd[HD : HD + 1, :],
                            recr[HD : HD + 1, :],
                            start=True, stop=True,
                        )
                        yraw = pC.tile([HD, QTL], F32, tag="yraw", bufs=2)
                        nc.vector.tensor_copy(yraw[:], ps_y[0:HD, :])
                        ynrm = pC.tile([HD, QTL], F32R, tag="ynrm", bufs=2)
                        nc.vector.tensor_tensor(
                            ynrm[:],
                            yraw[:],
                            ps_bc[0:HD, h * QTL : (h + 1) * QTL],
                            ALU.mult,
                        )
                        buf = a2a_in[t // NCORES]
                        nc.sync.dma_start(
                            buf[t % NCORES, h * HD : (h + 1) * HD, :], ynrm[:]
                        )
                    units.append(u_tail)
                return units

            def proj_units(half):
                """project the 8 gathered dim-blocks for a2a half `half`."""
                units = []
                yr_tiles = []

                def u_load():
                    for kk in range(NCORES):
                        yr = pD.tile([P, QTL], F32R, tag="yr", bufs=8)
                        nc.sync.dma_start(yr[:], a2a_out[half][kk])
                        yr_tiles.append(yr)
                units.append(u_load)

                for oc in range(CK):
                    def u_oc(oc=oc):
                        ps_o = psp.tile([P, QTL], F32, tag="qkv", bufs=2)
                        for kk in range(NCORES):
                            nc.tensor.matmul(
                                ps_o[:],
                                wpr[:, kk, oc * P : (oc + 1) * P],
                                yr_tiles[kk][:],
                                start=(kk == 0), stop=(kk == NCORES - 1),
                            )
                        osb = pD.tile([P, QTL], F32, tag="osb", bufs=2)
                        nc.vector.tensor_copy(osb[:], ps_o[:])
                        nc.scalar.dma_start(
                            outT[oc * P : (oc + 1) * P,
                                 half * QTL : (half + 1) * QTL],
                            osb[:],
                        )
                    units.append(u_oc)
                return units

            # =====================================================
            # emission
            # =====================================================
            for n in range(NT + 1):
                ub = b_units(n) if n < NT else []
                uc = c_units(n - 1) if n >= 1 else []
                if n == NT:
                    ub = proj_units(0)
                for u in _interleave(uc, ub):
                    u()
                if n - 1 == NCORES - 1:
                    nc.gpsimd.collective_compute(
                        "AllToAll", ALU.bypass, replica_groups=rg,
                        ins=[a2a_in[0][:].opt()], outs=[a2a_out[0][:].opt()],
                    )
                if n == 12:
                    nc.scalar.dma_start(
                        wpr[:], wp[:].rearrange("(ko p) d -> p ko d", p=P)
                    )

            nc.gpsimd.collective_compute(
                "AllToAll", ALU.bypass, replica_groups=rg,
                ins=[a2a_in[1][:].opt()], outs=[a2a_out[1][:].opt()],
            )
            for u in proj_units(1):
                u()

    nc.finalize()
    return nc


def _get_nc():
    global _CACHED_NC
    if _CACHED_NC is None:
        _CACHED_NC = _build_nc()
    return _CACHED_NC


def kernel(x, mask, w_qkv, w_proj):
    x = np.asarray(x, dtype=np.float32)
    mask = np.asarray(mask)
    w_qkv = np.asarray(w_qkv, dtype=np.float32)
    w_proj = np.ascontiguousarray(np.asarray(w_proj, dtype=np.float32))

    bf = ml_dtypes.bfloat16
    xf = np.ascontiguousarray(x.reshape(ROWS, C).astype(bf))
    # transposed diagonal-block mask patterns, duplicated for both heads
    mt = mask.reshape(T, T).T.astype(np.float32)
    maskp = np.stack([mt[m * P : m * P + P, 0:QTL] for m in range(4)])
    mask2 = np.ascontiguousarray(
        np.concatenate([maskp, maskp], axis=-1).astype(bf)
    )

    in_maps = []
    for r in range(NCORES):
        wq_s = (w_qkv[:, r * D2 : (r + 1) * D2] * SCALE).astype(bf)
        wk_s = w_qkv[:, C + r * D2 : C + (r + 1) * D2].astype(bf)
        wv_s = w_qkv[:, 2 * C + r * D2 : 2 * C + (r + 1) * D2].astype(bf)
        in_maps.append(
            {
                "x_full": xf,
                "wq": np.ascontiguousarray(wq_s),
                "wk": np.ascontiguousarray(wk_s),
                "wv": np.ascontiguousarray(wv_s),
                "wp": w_proj,
                "mask2": mask2,
            }
        )

    nc = _get_nc()
    res = run_bass_kernel_spmd(nc, in_maps, core_ids=list(range(NCORES)))
    global LAST_RESULTS
    LAST_RESULTS = res

    out = np.empty((ROWS, C), dtype=np.float32)
    for r in range(NCORES):
        oT = res.results[r]["outT"]
        out[r * QTL : (r + 1) * QTL, :] = oT[:, 0:QTL].T
        out[ROWS // 2 + r * QTL : ROWS // 2 + (r + 1) * QTL, :] = oT[:, QTL:].T
    return out.reshape(B, T, C)
